# revision 33
# baseline (speedup 1.0000x reference)
"""Trainium2 Bass kernel for nn_MILPAttention (dense multi-head attention with
per-key additive bias), tensor-parallel over heads across 8 NeuronCores.

Self-contained: hardcodes shapes N=4096, D=1024, H=16, GAMMA=1.0.

Math (reference):
    q = x @ Wq.T + bq ; k = x @ Wk.T + bk ; v = x @ Wv.T + bv     (per head, dh=64)
    logits = (q @ k.T) / 8 - h[key]
    attn = softmax(logits, keys)
    out = (attn @ v) @ Wo.T + bo + x

Per-core strategy (core i owns heads 2i, 2i+1 = columns 128i:128(i+1)):
    - Projections computed transposed: qT,kT [128, 4096] = W.T.T @ x.T with the
      1/8 scale folded into Wq/bq on the host. v computed transposed then
      PE-transposed to natural [keys, 64] layout and pre-scaled by w=exp(-h)
      (folds the per-key softmax bias into V); w appended as a 65th column so
      the P@V matmul also yields the softmax denominator.
    - S^T[key, q] = kT.T @ qT per head (K=64 contraction), exp on ScalarE
      (no max subtraction: logits are bounded ~ +-12), P kept bf16.
    - outT[d, q] (+denominator row) = vw.T @ P^T accumulated over key chunks.
    - Normalize by broadcasting 1/denominator, AllToAll to switch from
      head-sharding to sequence-sharding, then the output projection + bias
      + residual for this core's 512 rows.
"""
import numpy as np

import concourse.bass as bass
import concourse.mybir as mybir
import concourse.tile as tile
from concourse import bacc
from concourse.bass_utils import run_bass_kernel_spmd
from concourse.masks import make_identity

N, D, H = 4096, 1024, 16
NCORE = 8
CB = D // NCORE          # 128 columns (2 heads) per core
NR = N // NCORE          # 512 output rows per core
DH = D // H              # 64
KCH = N // 128           # 32 key chunks
NB = N // 512            # 8 n-blocks
BQ = 1024                # per-head q-block width in attention phase
QB = N // BQ             # 4 q-blocks
FP = mybir.dt.float32
BF = mybir.dt.bfloat16
AF = mybir.ActivationFunctionType


def _body(nc, tc, reps, xt, xr, wqt, wkt, wvt, wot, bqv, bkv, bvv, bov, hv, out,
          dbg=None, use_collective=True):
    cst = tc.alloc_tile_pool(name="cst", bufs=1)
    per = tc.alloc_tile_pool(name="per", bufs=1)
    dram = tc.alloc_tile_pool(name="dram", bufs=1, space="DRAM")

    ident = cst.tile([128, 128], BF)
    make_identity(nc, ident[:])

    # persistent sbuf
    wq_b = per.tile([128, D], BF)        # [d-in-chunk, dc*128 + c]
    wk_b = per.tile([128, D], BF)
    wv_b = per.tile([128, D], BF)
    wo_b = per.tile([128, 8 * D], BF)    # [c-in-chunk, cc*1024 + o]
    qb_t = per.tile([128, N], BF)        # qT: rows = 2 heads x 64 dims
    kb_t = per.tile([128, N], BF)
    vw_a = per.tile([128, KCH * 130], BF)  # per key chunk: 65 cols per head
    ao_s = per.tile([128, N], BF)        # normalized attn-out^T
    bq_s = per.tile([128, 1], FP)
    bk_s = per.tile([128, 1], FP)
    bv_s = per.tile([128, 1], FP)
    w_s = per.tile([128, KCH], FP)       # exp(-h), [key-in-chunk, chunk]
    xb_s = [per.tile([128, D], FP, name=f"xb{j}") for j in range(4)]  # x rows + bo

    cc_in = dram.tile([NCORE * 128, NR], BF)
    cc_out = dram.tile([NCORE * 128, NR], BF)

    for rep in range(reps):
        sfx = f"_{rep}"
        # ---------------- phase 0: constants ----------------
        with tc.tile_pool(name="p0" + sfx, bufs=2) as p0:
            for wi, (wsrc, wdst) in enumerate(
                    ((wqt, wq_b), (wkt, wk_b), (wvt, wv_b))):
                eng = (nc.sync, nc.scalar, nc.gpsimd)[wi]
                # [D, CB] bf16 -> [128, dc*128 + c] in one rearranged DMA
                eng.dma_start(wdst[:].rearrange("p (dc c) -> p dc c", c=CB),
                              wsrc.rearrange("(dc p) c -> p dc c", p=128))
            nc.gpsimd.dma_start(bq_s[:], bqv.unsqueeze(1))
            nc.gpsimd.dma_start(bk_s[:], bkv.unsqueeze(1))
            nc.gpsimd.dma_start(bv_s[:], bvv.unsqueeze(1))
            hst = p0.tile([128, KCH], FP, name="hst")
            nc.gpsimd.dma_start(hst[:], hv.rearrange("(c p) -> p c", p=128))
            nc.scalar.activation(w_s[:], hst[:], AF.Exp, scale=-1.0)

        # ---------------- phase 1a: k and v projections ----------------
        with tc.tile_pool(name="p1s" + sfx, bufs=6) as p1s, \
             tc.tile_pool(name="p1p", bufs=2, space="PSUM") as p1p, \
             tc.tile_pool(name="p1t", bufs=2, space="PSUM") as p1t:
            for nb in range(NB):
                xtb = []
                for dc in range(8):
                    xb = p1s.tile([128, 512], BF, name="xb")
                    (nc.sync if dc % 2 == 0 else nc.scalar).dma_start(
                        xb[:], xt[dc * 128:(dc + 1) * 128, nb * 512:(nb + 1) * 512])
                    xtb.append(xb)
                psk = p1p.tile([128, 512], FP, name="psk")
                psv = p1p.tile([128, 512], FP, name="psv")
                for dc in range(8):
                    st, sp = dc == 0, dc == 7
                    nc.tensor.matmul(psk[:], wk_b[:, dc * CB:(dc + 1) * CB], xtb[dc][:],
                                     start=st, stop=sp)
                    nc.tensor.matmul(psv[:], wv_b[:, dc * CB:(dc + 1) * CB], xtb[dc][:],
                                     start=st, stop=sp)
                ncol = slice(nb * 512, (nb + 1) * 512)
                nc.vector.tensor_scalar_add(kb_t[:, ncol], psk[:], bk_s[:, 0:1])
                vtb = p1s.tile([128, 512], BF, name="vtb")
                nc.vector.tensor_scalar_add(vtb[:], psv[:], bv_s[:, 0:1])
                for ns in range(4):
                    kc = nb * 4 + ns
                    pvt = p1t.tile([128, 128], BF, name="pvt")
                    nc.tensor.transpose(pvt[:], vtb[:, ns * 128:(ns + 1) * 128], ident[:])
                    c0 = kc * 130
                    nc.vector.tensor_scalar_mul(
                        vw_a[:, c0:c0 + 64], pvt[:, 0:64], w_s[:, kc:kc + 1])
                    nc.vector.tensor_copy(vw_a[:, c0 + 64:c0 + 65], w_s[:, kc:kc + 1])
                    nc.vector.tensor_scalar_mul(
                        vw_a[:, c0 + 65:c0 + 129], pvt[:, 64:128], w_s[:, kc:kc + 1])
                    nc.vector.tensor_copy(vw_a[:, c0 + 129:c0 + 130], w_s[:, kc:kc + 1])

        # ------- phase 1b (q projection) + phase 2 (attention), overlapped ----
        # pools held open together: psq 1 + pss 2x2 + pso 2 = 7 psum banks, so
        # the scheduler is free to run the q projection under early attention.
        with tc.tile_pool(name="p1bs" + sfx, bufs=4) as p1bs, \
             tc.tile_pool(name="p1bq", bufs=1, space="PSUM") as p1bq, \
             tc.tile_pool(name="p2s" + sfx, bufs=3) as p2s, \
             tc.tile_pool(name="p2n", bufs=2) as p2n, \
             tc.tile_pool(name="p2ps", bufs=2, space="PSUM") as p2ps, \
             tc.tile_pool(name="p2po", bufs=1, space="PSUM") as p2po:
            for nb in range(NB):
                xtb = []
                for dc in range(8):
                    xb2 = p1bs.tile([128, 512], BF, name="xb2")
                    (nc.sync if dc % 2 == 0 else nc.scalar).dma_start(
                        xb2[:], xt[dc * 128:(dc + 1) * 128, nb * 512:(nb + 1) * 512])
                    xtb.append(xb2)
                psq = p1bq.tile([128, 512], FP, name="psq")
                for dc in range(8):
                    nc.tensor.matmul(psq[:], wq_b[:, dc * CB:(dc + 1) * CB], xtb[dc][:],
                                     start=(dc == 0), stop=(dc == 7))
                nc.vector.tensor_scalar_add(qb_t[:, nb * 512:(nb + 1) * 512],
                                            psq[:], bq_s[:, 0:1])

            for qb in range(QB):
                qcol = slice(qb * BQ, (qb + 1) * BQ)
                for h in range(2):
                    hr = slice(h * 64, (h + 1) * 64)
                    pso = p2po.tile([65, BQ], FP, name="pso")
                    for kc in range(KCH):
                        krng = slice(kc * 128, (kc + 1) * 128)
                        pss = p2ps.tile([128, BQ], FP, name="pss")
                        for j in range(BQ // 512):
                            nc.tensor.matmul(
                                pss[:, j * 512:(j + 1) * 512],
                                kb_t[hr, krng],
                                qb_t[hr, qb * BQ + j * 512: qb * BQ + (j + 1) * 512],
                                start=True, stop=True)
                        pb = p2s.tile([128, BQ], BF, name="pb")
                        nc.scalar.activation(pb[:], pss[:], AF.Exp)
                        lh = vw_a[:, kc * 130 + 65 * h: kc * 130 + 65 * h + 65]
                        for j in range(BQ // 512):
                            nc.tensor.matmul(
                                pso[:, j * 512:(j + 1) * 512], lh,
                                pb[:, j * 512:(j + 1) * 512],
                                start=(kc == 0), stop=(kc == KCH - 1))
                    # snap pso to SBUF fast (pso is single-buffered), then
                    # normalize from the copy: ao = snap[0:64]/snap[64]
                    snap = p2n.tile([65, BQ], FP, name="snap")
                    nc.vector.tensor_copy(snap[:], pso[:])
                    rc = p2n.tile([1, BQ], FP, name=f"rc{h}")
                    nc.vector.reciprocal(rc[:], snap[64:65, :])
                    bch = p2n.tile([64, BQ], FP, name=f"bc{h}")
                    nc.gpsimd.partition_broadcast(bch[:], rc[:])
                    nc.vector.tensor_mul(ao_s[hr, qcol], snap[0:64, :], bch[:])

        # prefetch phase-3 constants: queued behind the projection DMAs so the
        # transfers land during phase 2 (DMA is idle there)
        with tc.tile_pool(name="pf" + sfx, bufs=2) as pf:
            nc.sync.dma_start(wo_b[:].rearrange("p (cc o) -> p cc o", o=D),
                              wot.rearrange("(cc p) o -> p cc o", p=128))
            bost = pf.tile([128, D], FP, name="bost")
            nc.sync.dma_start(bost[:], bov.unsqueeze(0).broadcast_to([128, D]))
            for j in range(4):
                xrt = pf.tile([128, D], FP, name="xrt")
                (nc.sync if j % 2 == 0 else nc.scalar).dma_start(
                    xrt[:], xr[j * 128:(j + 1) * 128, :])
                nc.vector.tensor_add(xb_s[j][:], xrt[:], bost[:])

        # ---------------- phase 3: A2A + out projection ----------------
        with tc.tile_pool(name="p3s" + sfx, bufs=2) as p3s, \
             tc.tile_pool(name="p3p", bufs=1, space="PSUM") as p3p:
            for j in range(NCORE):
                nc.sync.dma_start(cc_in[j * 128:(j + 1) * 128, :],
                                  ao_s[:, j * NR:(j + 1) * NR])
            if use_collective:
                nc.gpsimd.collective_compute(
                    "AllToAll", mybir.AluOpType.bypass,
                    replica_groups=[list(range(NCORE))],
                    ins=[cc_in[:].opt()], outs=[cc_out[:].opt()])
            else:  # single-core timing-sim stand-in
                nc.sync.dma_start(cc_out[:], cc_in[:])
            psf = [p3p.tile([128, 512], FP, name=f"psf{t}") for t in range(8)]
            for cc in range(8):
                aoc = p3s.tile([128, NR], BF, name="aoc")
                nc.sync.dma_start(aoc[:], cc_out[cc * 128:(cc + 1) * 128, :])
                if dbg is not None:
                    nc.sync.dma_start(dbg["d_aoc"][cc * 128:(cc + 1) * 128, :], aoc[:])
                for ns in range(4):
                    for ob in range(2):
                        nc.tensor.matmul(
                            psf[ns * 2 + ob][:],
                            aoc[:, ns * 128:(ns + 1) * 128],
                            wo_b[:, cc * D + ob * 512: cc * D + (ob + 1) * 512],
                            start=(cc == 0), stop=(cc == 7))
            for ns in range(4):
                for ob in range(2):
                    fo = p3s.tile([128, 512], FP, name="fo")
                    nc.vector.tensor_add(fo[:], psf[ns * 2 + ob][:],
                                         xb_s[ns][:, ob * 512:(ob + 1) * 512])
                    nc.sync.dma_start(
                        out[ns * 128:(ns + 1) * 128, ob * 512:(ob + 1) * 512], fo[:])

    if dbg is not None:
        for nm, t in (("d_q", qb_t), ("d_k", kb_t), ("d_vw", vw_a), ("d_ao", ao_s)):
            nc.sync.dma_start(dbg[nm], t[:])
        nc.sync.dma_start(dbg["d_cc"], cc_out[:])

    dram.release()
    per.release()
    cst.release()


def build_nc(reps=1, debug=False, use_collective=True):
    nc = bacc.Bacc("TRN2", target_bir_lowering=False, debug=False, num_devices=NCORE)
    xt = nc.dram_tensor("xt", [D, N], BF, kind="ExternalInput").ap()
    xr = nc.dram_tensor("xr", [NR, D], FP, kind="ExternalInput").ap()
    wqt = nc.dram_tensor("wqt", [D, CB], BF, kind="ExternalInput").ap()
    wkt = nc.dram_tensor("wkt", [D, CB], BF, kind="ExternalInput").ap()
    wvt = nc.dram_tensor("wvt", [D, CB], BF, kind="ExternalInput").ap()
    wot = nc.dram_tensor("wot", [D, D], BF, kind="ExternalInput").ap()
    bqv = nc.dram_tensor("bqv", [CB], FP, kind="ExternalInput").ap()
    bkv = nc.dram_tensor("bkv", [CB], FP, kind="ExternalInput").ap()
    bvv = nc.dram_tensor("bvv", [CB], FP, kind="ExternalInput").ap()
    bov = nc.dram_tensor("bov", [D], FP, kind="ExternalInput").ap()
    hv = nc.dram_tensor("hv", [N], FP, kind="ExternalInput").ap()
    out = nc.dram_tensor("out", [NR, D], FP, kind="ExternalOutput").ap()
    dbg = None
    if debug:
        dbg = {
            "d_q": nc.dram_tensor("d_q", [128, N], BF, kind="ExternalOutput").ap(),
            "d_k": nc.dram_tensor("d_k", [128, N], BF, kind="ExternalOutput").ap(),
            "d_vw": nc.dram_tensor("d_vw", [128, KCH * 130], BF, kind="ExternalOutput").ap(),
            "d_ao": nc.dram_tensor("d_ao", [128, N], BF, kind="ExternalOutput").ap(),
            "d_cc": nc.dram_tensor("d_cc", [NCORE * 128, NR], BF, kind="ExternalOutput").ap(),
            "d_aoc": nc.dram_tensor("d_aoc", [NCORE * 128, NR], BF, kind="ExternalOutput").ap(),
        }
    with tile.TileContext(nc) as tc:
        _body(nc, tc, reps, xt, xr, wqt, wkt, wvt, wot,
              bqv, bkv, bvv, bov, hv, out, dbg=dbg, use_collective=use_collective)
    nc.compile()
    return nc


_NC_CACHE = {}


def get_nc(reps=1):
    if reps not in _NC_CACHE:
        _NC_CACHE[reps] = build_nc(reps)
    return _NC_CACHE[reps]


def make_in_maps(inputs):
    x = np.ascontiguousarray(np.asarray(inputs["x"], dtype=np.float32))
    h = np.ascontiguousarray(np.asarray(inputs["h"], dtype=np.float32))
    Wq = np.asarray(inputs["Wq"], dtype=np.float32)
    bq = np.asarray(inputs["bq"], dtype=np.float32)
    Wk = np.asarray(inputs["Wk"], dtype=np.float32)
    bk = np.asarray(inputs["bk"], dtype=np.float32)
    Wv = np.asarray(inputs["Wv"], dtype=np.float32)
    bv = np.asarray(inputs["bv"], dtype=np.float32)
    Wo = np.asarray(inputs["Wo"], dtype=np.float32)
    bo = np.ascontiguousarray(np.asarray(inputs["bo"], dtype=np.float32))
    import ml_dtypes
    bf16 = ml_dtypes.bfloat16
    xt = np.ascontiguousarray(x.T.astype(bf16))
    wot = np.ascontiguousarray(Wo.T.astype(bf16))
    scale = np.float32(0.125)  # 1/sqrt(dh), folded into q
    in_maps = []
    for i in range(NCORE):
        cs = slice(i * CB, (i + 1) * CB)
        in_maps.append({
            "xt": xt,
            "xr": np.ascontiguousarray(x[i * NR:(i + 1) * NR, :]),
            "wqt": np.ascontiguousarray((Wq[cs, :] * scale).T.astype(bf16)),
            "wkt": np.ascontiguousarray(Wk[cs, :].T.astype(bf16)),
            "wvt": np.ascontiguousarray(Wv[cs, :].T.astype(bf16)),
            "wot": wot,
            "bqv": np.ascontiguousarray(bq[cs] * scale),
            "bkv": np.ascontiguousarray(bk[cs]),
            "bvv": np.ascontiguousarray(bv[cs]),
            "bov": bo,
            "hv": h,
        })
    return in_maps


def kernel(**inputs):
    nc = get_nc(1)
    in_maps = make_in_maps(inputs)
    res = run_bass_kernel_spmd(nc, in_maps, core_ids=list(range(NCORE)))
    return np.concatenate([res.results[i]["out"] for i in range(NCORE)], axis=0)



